# revision 20
# baseline (speedup 1.0000x reference)
"""Trainium2 Bass kernel for nn_CapsuleConv_4595615007178.

Math (reference): for input x[B,N,L,D], weights w[K,N,4,4,M]:
  nv[b,m,w,a,d] = (1/M) * sum_{n,k,x} x[b,n,w+k,a*4+x] * w[k,n,x,d,m]
  out = LayerNorm_{(a,d)}(nv) * gamma + beta      (eps=1e-5)

Device mapping (per core, data-parallel over batch, 2 batches/core):
  For each (b, a): V_a[(m,d), w] = sum_k Wk[(n,x),(m,d)]^T @ X_a[(n,x), w+k]
  - contraction (n,x)=128 on partitions, sliding window k = free-dim offset
  - fp16 inputs/weights (halves HBM traffic), fp32 PSUM accumulation
  - LayerNorm stats via broadcast-fused PE reduction: lhsT = kron(I_32,
    ones(4,4))/16 accumulated over the 4 a-tiles gives mean (and E[v^2])
    already broadcast to all 128 (m,d) partitions - no separate bcast pass.
  - normalize on DVE/Pool in fp16 (DVE 2x mode), y written as fp16.
  - software-pipelined emission: stats matmuls lag 1 unit, normalize lags
    3 units, so no engine queue head-blocks on a cross-engine dep chain.
Host side does layout prep + dtype casts only (pure permutations), no FLOPs.
"""

import numpy as np

# problem dims (hardcoded per contest contract)
B, N, L, D = 16, 32, 2048, 16
M, OUT_D = 32, 16
K = 3
A, SQ = 4, 4
W = 2046
NCORES = 8
BPC = B // NCORES  # batches per core
CHUNKS = [(0, 512), (512, 512), (1024, 512), (1536, 510)]
NCH = len(CHUNKS)
EPS = 1e-5

_CACHE = {}


def _build(
    apply_gb=False, reps=1, chain=False, timing=False, ncores=NCORES,
    io_mode="sp", no_io=False, loopn=1,
):
    import concourse.bacc as bacc
    import concourse.mybir as mybir
    from concourse import tile

    f16 = mybir.dt.float16
    f32 = mybir.dt.float32
    AL = mybir.AluOpType

    nc = bacc.Bacc("TRN2", target_bir_lowering=False, debug=False, num_devices=ncores)
    xt = nc.dram_tensor("x", [BPC, A, 128, L], f16, kind="ExternalInput")
    wt = nc.dram_tensor("w", [K, 128, 128], f16, kind="ExternalInput")
    bst = nc.dram_tensor("bsmat", [128, 128], f16, kind="ExternalInput")
    if apply_gb:
        gbt = nc.dram_tensor("gb", [2, A, 128, 1], f32, kind="ExternalInput")
    yt = nc.dram_tensor(
        "y", [BPC, A, 128, W], f16, kind="Internal" if timing else "ExternalOutput"
    )
    if timing:
        tick = nc.dram_tensor("tick", [128, 1], f16, kind="ExternalOutput")

    NU = BPC * NCH          # units per rep (8)
    NB = reps * BPC         # total batch-groups
    TOT = reps * NU         # total units
    LAG_S = 1               # stats-matmul lag (units)
    LAG_N = 3               # normalize lag (units)

    with tile.TileContext(nc) as tc:
        with (
            tc.tile_pool(name="consts", bufs=1) as cpool,
            tc.tile_pool(name="xin", bufs=2) as xpool,
            tc.tile_pool(name="vbuf", bufs=4) as vpool,
            tc.tile_pool(name="sqbuf", bufs=3) as sqpool,
            tc.tile_pool(name="stat", bufs=2) as stpool,
            tc.tile_pool(name="obuf", bufs=2) as opool,
            tc.tile_pool(name="pmain", bufs=1, space="PSUM") as pmain,
            tc.tile_pool(name="pstat", bufs=2, space="PSUM") as pstat,
        ):
            w_sb = cpool.tile([128, K * 128], f16, name="w_sb")
            for k in range(K):
                nc.sync.dma_start(w_sb[:, k * 128 : (k + 1) * 128], wt[k])
            bs_sb = cpool.tile([128, 128], f16, name="bs_sb")
            nc.sync.dma_start(bs_sb[:], bst[:])
            if apply_gb:
                gb_sb = cpool.tile([128, 2 * A], f32, name="gb_sb")
                for a in range(A):
                    nc.sync.dma_start(gb_sb[:, a : a + 1], gbt[0, a])
                    nc.sync.dma_start(gb_sb[:, A + a : A + a + 1], gbt[1, a])

            st = {}      # unit index -> stage state
            xg = {}      # batch-group -> x tiles
            og = {}      # batch-group -> o tiles
            import contextlib

            loop_cm = tc.For_i(0, loopn, 1) if loopn > 1 else contextlib.nullcontext()

            if io_mode == "sp":
                ld_engines = st_engines = [nc.sync] * 4
            elif io_mode == "split":
                ld_engines = [nc.sync] * 4
                st_engines = [nc.gpsimd] * 4
            elif io_mode == "spread":
                ld_engines = [nc.sync, nc.gpsimd, nc.sync, nc.gpsimd]
                st_engines = [nc.gpsimd, nc.sync, nc.gpsimd, nc.sync]

            def load_x(g):
                rep, b = divmod(g, BPC)
                if no_io and rep > 0:
                    xg[g] = xg[g - BPC]
                    return
                tiles = []
                for a in range(A):
                    xa = xpool.tile(
                        [128, L], f16, tag=f"x{a}", name=f"x_{g}_{a}"
                    )
                    ld_engines[a].dma_start(xa[:], xt[b, a])
                    tiles.append(xa)
                xg[g] = tiles

            with loop_cm:
              for r in range(TOT + LAG_N + 1):
                # ---------- stage MAIN: unit r ----------
                if r < TOT:
                    rep, u = divmod(r, NU)
                    b, ci = divmod(u, NCH)
                    g = rep * BPC + b
                    c0, cw = CHUNKS[ci]
                    if r == 0:
                        load_x(0)
                    if ci == 0 and g + 1 < NB:
                        load_x(g + 1)  # prefetch next batch-group
                    x_sb = xg[g]
                    ent = st[r] = {"meta": (rep, b, ci, c0, cw, g)}
                    # main matmuls: k-outer (3 stationary-weight loads/unit)
                    pv = [
                        pmain.tile(
                            [128, cw], f32, tag=f"pv{a}", bufs=1, name=f"pv_{r}_{a}"
                        )
                        for a in range(A)
                    ]
                    for k in range(K):
                        for a in range(A):
                            nc.tensor.matmul(
                                pv[a][:],
                                lhsT=w_sb[:, k * 128 : (k + 1) * 128],
                                rhs=x_sb[a][:, k + c0 : k + c0 + cw],
                                start=(k == 0),
                                stop=(k == K - 1),
                            )
                    # Act: evacuate PSUM -> SBUF fp16 (frees pv banks)
                    v = [
                        vpool.tile([128, cw], f16, tag=f"v{a}", name=f"v_{r}_{a}")
                        for a in range(A)
                    ]
                    for a in range(A):
                        nc.scalar.copy(v[a][:], pv[a][:])
                    # DVE: squares (fp16 2x mode)
                    sq = [
                        sqpool.tile([128, cw], f16, tag=f"sq{a}", name=f"sq_{r}_{a}")
                        for a in range(A)
                    ]
                    for a in range(A):
                        nc.vector.tensor_tensor(sq[a][:], v[a][:], v[a][:], op=AL.mult)
                    ent["v"] = v
                    ent["sq"] = sq

                # ---------- Act sqrt for unit r-LAG_N (dep always ready;
                # emit before musq/mub so it doesn't sit behind their
                # mid-window pmub dependency in the Act queue) ----------
                s = r - LAG_N
                if 0 <= s < TOT:
                    ent = st[s]
                    _, _, _, _, cw, _ = ent["meta"]
                    rstdb = stpool.tile([128, cw], f16, tag="rstdb", name=f"rstdb_{s}")
                    nc.scalar.sqrt(rstdb[:], ent["rv"][:])
                    ent["rstdb"] = rstdb

                # ---------- stage STATS-MM: unit r-LAG_S ----------
                s = r - LAG_S
                if 0 <= s < TOT:
                    ent = st[s]
                    _, _, _, _, cw, _ = ent["meta"]
                    pmub = pstat.tile([128, cw], f32, tag="pmub", name=f"pmub_{s}")
                    pqb = pstat.tile([128, cw], f32, tag="pqb", name=f"pqb_{s}")
                    for a in range(A):
                        nc.tensor.matmul(
                            pmub[:],
                            lhsT=bs_sb[:],
                            rhs=ent["v"][a][:],
                            start=(a == 0),
                            stop=(a == A - 1),
                        )
                    for a in range(A):
                        nc.tensor.matmul(
                            pqb[:],
                            lhsT=bs_sb[:],
                            rhs=ent["sq"][a][:],
                            start=(a == 0),
                            stop=(a == A - 1),
                        )
                    ent["pmub"] = pmub
                    ent["pqb"] = pqb
                    # Act: mean broadcast -> SBUF fp16 (Pool cannot read PSUM)
                    mub = stpool.tile([128, cw], f16, tag="mub", name=f"mub_{s}")
                    nc.scalar.copy(mub[:], pmub[:])
                    ent["mub"] = mub
                    # Act: musq = mean^2
                    musq = stpool.tile([128, cw], f32, tag="musq", name=f"musq_{s}")
                    nc.scalar.square(musq[:], pmub[:])
                    ent["musq"] = musq

                # ---------- stage VAR: unit r-LAG_S-1 ----------
                s = r - LAG_S - 1
                if 0 <= s < TOT:
                    ent = st[s]
                    _, _, _, _, cw, _ = ent["meta"]
                    # DVE: vare = (pqb + eps) - musq  (stt only exists on DVE)
                    vare = stpool.tile([128, cw], f32, tag="vare", name=f"vare_{s}")
                    nc.vector.scalar_tensor_tensor(
                        vare[:], ent["pqb"][:], EPS, ent["musq"][:],
                        op0=AL.add, op1=AL.subtract,
                    )
                    ent["vare"] = vare

                # ---------- stage RSTD: unit r-LAG_S-1 (DVE) / r-LAG_N+1 (Act) ----
                s = r - LAG_S - 1
                if 0 <= s < TOT:
                    ent = st[s]
                    _, _, _, _, cw, _ = ent["meta"]
                    rv = stpool.tile([128, cw], f32, tag="rv", name=f"rv_{s}")
                    nc.vector.reciprocal(rv[:], ent["vare"][:])
                    ent["rv"] = rv
                # ---------- stage NORM: unit r-LAG_N ----------
                s = r - LAG_N
                if 0 <= s < TOT:
                    ent = st[s]
                    rep, b, ci, c0, cw, g = ent["meta"]
                    if ci == 0:
                        og[g] = [
                            opool.tile([128, W], f16, tag=f"o{a}", name=f"o_{g}_{a}")
                            for a in range(A)
                        ]
                    o = og[g]
                    mub, rstdb = ent["mub"], ent["rstdb"]
                    for a in range(A):
                        osl = o[a][:, c0 : c0 + cw]
                        sub_eng = nc.vector if a < 2 else nc.gpsimd
                        mul_eng = nc.vector if a < 3 else nc.gpsimd
                        sub_eng.tensor_tensor(osl, ent["v"][a][:], mub[:], op=AL.subtract)
                        mul_eng.tensor_tensor(osl, osl, rstdb[:], op=AL.mult)
                        if apply_gb:
                            nc.vector.tensor_scalar(
                                osl, osl, gb_sb[:, a : a + 1],
                                gb_sb[:, A + a : A + a + 1],
                                op0=AL.mult, op1=AL.add,
                            )
                    # light cross-rep serialization for the timing harness
                    if chain and rep > 0 and b == 0 and ci == 0:
                        ylook = stpool.tile(
                            [128, 1], f16, tag="ylook", bufs=2, name=f"ylook_{rep}"
                        )
                        nc.sync.dma_start(ylook[:], yt[0, 0][:, 0:1])
                        zl = stpool.tile(
                            [128, 1], f16, tag="zl", bufs=2, name=f"zl_{rep}"
                        )
                        nc.vector.tensor_scalar(zl[:], ylook[:], 0.0, None, AL.mult)
                        nc.vector.tensor_tensor(
                            o[0][:, 0:1], o[0][:, 0:1], zl[:], op=AL.add
                        )
                    if ci == NCH - 1 and not no_io:
                        for a in range(A):
                            st_engines[a].dma_start(yt[b, a], o[a][:])
                    # drop refs no longer needed
                    del st[s]

              if timing:
                tk = stpool.tile([128, 1], f16, tag="tick", bufs=1, name="tk")
                nc.scalar.copy(tk[:], og[NB - 1][0][:, 0:1])
                nc.sync.dma_start(tick[:, :], tk[:])

    nc.compile()
    return nc


def _get_nc(apply_gb):
    key = ("nc", apply_gb)
    if key not in _CACHE:
        _CACHE[key] = _build(apply_gb)
    return _CACHE[key]


def _prep(x, w):
    xp = (
        np.ascontiguousarray(x.reshape(B, N, L, A, SQ).transpose(0, 3, 1, 4, 2))
        .reshape(B, A, 128, L)
        .astype(np.float16)
    )
    wp = (
        np.ascontiguousarray((w / float(M)).transpose(0, 1, 2, 4, 3))
        .reshape(K, 128, 128)
        .astype(np.float16)
    )
    bs = (np.kron(np.eye(M), np.ones((SQ, SQ))) / float(OUT_D)).astype(np.float16)
    return xp, wp, bs


def _in_maps(xp, wp, bs, gamma=None, beta=None):
    maps = []
    for c in range(NCORES):
        m = {"x": xp[c * BPC : (c + 1) * BPC], "w": wp, "bsmat": bs}
        if gamma is not None:
            gb = np.empty((2, A, 128, 1), np.float32)
            for a in range(A):
                gb[0, a, :, 0] = np.tile(gamma.reshape(A, SQ)[a], M)
                gb[1, a, :, 0] = np.tile(beta.reshape(A, SQ)[a], M)
            m["gb"] = gb
        maps.append(m)
    return maps


def kernel(x, w, gamma, beta, num_iter=None, **_unused):
    from concourse.bass_utils import run_bass_kernel_spmd

    x = np.asarray(x, dtype=np.float32)
    w = np.asarray(w, dtype=np.float32)
    gamma = np.asarray(gamma, dtype=np.float32)
    beta = np.asarray(beta, dtype=np.float32)

    apply_gb = not (np.all(gamma == 1.0) and np.all(beta == 0.0))

    xp, wp, bs = _prep(x, w)
    nc = _get_nc(apply_gb)
    maps = _in_maps(xp, wp, bs, gamma if apply_gb else None, beta if apply_gb else None)

    res = run_bass_kernel_spmd(nc, maps, list(range(NCORES)))
    y = np.stack([res.results[c]["y"] for c in range(NCORES)])  # [8, BPC, A, 128, W]
    y = (
        y.astype(np.float32)
        .reshape(B, A, M, SQ, W)
        .transpose(0, 2, 4, 1, 3)
        .reshape(B, M, W, OUT_D)
    )
    return np.ascontiguousarray(y)
